# revision 43
# baseline (speedup 1.0000x reference)
"""Trainium2 Bass kernel for a DEQ Transformer-XL layer (relative attention + FFN).

Sharding (8 cores): 2 batch groups x 4-way tensor parallel.
  core c: batch b = c//4, group g = c%4
  - qkv/r projections + attention: heads [4g, 4g+4) (Megatron column split)
  - o_net: partial over this core's 256 channels -> ReduceScatter(token-sharded)
  - LN on this core's 256-token slice -> token-local FFN (full d_inner, weights
    streamed/prefetched) -> final LN on token slice.

v2 design notes:
  - Scores are built TRANSPOSED (j on partitions): the rel-shift BD term is
    read from DRAM scratch with the efficient [i-part, j-free] diagonal AP,
    then injected into the S^T PSUM via a transpose-matmul (lhsT=bd block,
    rhs=identity), and the AC term accumulates on top (start=False).  exp's
    output (probT) then feeds the AV matmul directly -- no per-iteration
    XBAR transposes, no elementwise add, no prob renormalization pass.
  - Softmax row-sums come free from the AV matmul via a ones-augmented V
    (vTx has a constant-1 channel per head); normalization is deferred to a
    per-chunk [2,128]x[2-part] broadcast matmul + in-place column scale.
  - o_net runs per 128-token chunk; chunks are processed [0,2,4,6,1,3,5,7]
    so the first ReduceScatter (first token-half of each dest rank) launches
    ~60% through attention and the second right at the end; the FFN runs in
    token halves so ff1(half0) overlaps RS#2.
  - dscr scratch stores only the 1000 valid columns (pads NEG-filled once).
"""

from contextlib import ExitStack

import numpy as np
import ml_dtypes

import concourse.bass as bass
import concourse.mybir as mybir
import concourse.tile as tile
from concourse import bacc
from concourse import bass_utils

F32 = mybir.dt.float32
BF16 = mybir.dt.bfloat16
AF = mybir.ActivationFunctionType

B, D, Q, M = 2, 1024, 1024, 1024
K = Q + M                 # 2048
NHL = 4                   # heads per core
DH = 64
CO = NHL * DH             # 256 channels per core
DIL = 4096                # full d_inner (token-local FFN)
TL = 256                  # token slice per core (within batch group)
HT = 128                  # half-token slice (FFN pipelining)
SPAN = 1152               # aligned j-span per 128-query chunk
NJC = SPAN // 128         # 9
PREW = 1312               # padded pre-matrix width in DRAM
PREOFF = 153              # pre data placement within the padded row
RDOFF = 128               # diagonal read base: PREOFF - (M - LOCAL_SIZE)
EPS = 1e-5
DEBUG = False
NEG = -1e30
RG = [[0, 1, 2, 3], [4, 5, 6, 7]]
N_CORES = 8
CORDER = [0, 2, 4, 6, 1, 3, 5, 7]

_CACHE = {}


def _build():
    nc = bacc.Bacc("TRN2", target_bir_lowering=False, debug=False,
                   num_devices=N_CORES)

    def din(name, shape, dt=BF16):
        return nc.dram_tensor(name, shape, dt, kind="ExternalInput").ap()

    cat = din("cat", [D, K])
    wqT = din("wqT", [D, CO])
    wkT = din("wkT", [D, CO])
    wvT = din("wvT", [D, CO])
    wrT = din("wrT", [D, CO])
    u_q = din("u_q", [CO, Q])
    u_k = din("u_k", [CO, K])
    u_v = din("u_v", [CO, K])
    pos_w = din("pos_w", [D, 1000])
    ident_b = din("ident_b", [128, 128], BF16)
    ones_b = din("ones_b", [128, 1], BF16)
    onesr = din("onesr", [1, 128], F32)
    rwb = din("rwb", [128, 2], F32)
    rrb = din("rrb", [128, 2], F32)
    sel2 = din("sel2", [2, 128], BF16)
    woT = din("woT", [CO, D], BF16)
    zres = din("zres", [D, TL], F32)
    wff1T = din("wff1T", [D, DIL], BF16)
    bff1 = din("bff1", [128, 32], F32)
    wff2T = din("wff2T", [DIL, D], BF16)
    bff2 = din("bff2", [128, 8], F32)

    y_out = nc.dram_tensor("y", [D, TL], F32, kind="ExternalOutput").ap()
    if DEBUG:
        dbg_bd = nc.dram_tensor("dbg_bd", [128, 2, SPAN], BF16,
                                kind="ExternalOutput").ap()
        dbg_pt = nc.dram_tensor("dbg_pt", [128, NJC, 128], BF16,
                                kind="ExternalOutput").ap()
        dbg_av = nc.dram_tensor("dbg_av", [128, Q], BF16,
                                kind="ExternalOutput").ap()
        dbg_xat = nc.dram_tensor("dbg_xat", [128, 8, TL], F32,
                                 kind="ExternalOutput").ap()
        dbg_ri = nc.dram_tensor("dbg_ri", [1, 2, 128], BF16,
                                kind="ExternalOutput").ap()
        dbg_avp = nc.dram_tensor("dbg_avp", [65, 128], F32,
                                 kind="ExternalOutput").ap()
        dbg_vtx = nc.dram_tensor("dbg_vtx", [128, K // 128, 80], BF16,
                                 kind="ExternalOutput").ap()

    with tile.TileContext(nc) as tc, ExitStack() as stack:
        consts = stack.enter_context(tc.tile_pool(name="consts", bufs=1))
        id_b = consts.tile([128, 128], BF16)
        nc.gpsimd.dma_start(out=id_b, in_=ident_b)
        ones_sb = consts.tile([128, 1], BF16)
        nc.gpsimd.dma_start(out=ones_sb, in_=ones_b)
        onesr_sb = consts.tile([1, 128], F32)
        nc.gpsimd.dma_start(out=onesr_sb, in_=onesr)
        rwb_sb = consts.tile([128, 2], F32)
        nc.gpsimd.dma_start(out=rwb_sb, in_=rwb)
        rrb_sb = consts.tile([128, 2], F32)
        nc.gpsimd.dma_start(out=rrb_sb, in_=rrb)
        sel2_sb = consts.tile([2, 128], BF16)
        nc.gpsimd.dma_start(out=sel2_sb, in_=sel2)
        bff1_sb = consts.tile([128, 32], F32)
        nc.gpsimd.dma_start(out=bff1_sb, in_=bff1)
        bff2_sb = consts.tile([128, 8], F32)
        nc.gpsimd.dma_start(out=bff2_sb, in_=bff2)
        eps_sb = consts.tile([1, 1], F32)
        nc.vector.memset(eps_sb, EPS)
        neg_sb = consts.tile([128, 160], BF16)
        nc.vector.memset(neg_sb, NEG)

        # FFN w1 weights: prefetched during attention (8 MB)
        w1p = stack.enter_context(tc.tile_pool(name="w1p", bufs=1))
        w1_sb = [w1p.tile([128, DIL], BF16, tag=f"w1_{dc}", name=f"w1_{dc}")
                 for dc in range(8)]

        dram = stack.enter_context(tc.tile_pool(name="dram", bufs=4, space="DRAM"))
        rs1a_in = dram.tile([4, D, HT], BF16, tag="rs1ai")
        rs1a_out = dram.tile([D, HT], BF16, tag="rs1ao")
        rs1b_in = dram.tile([4, D, HT], BF16, tag="rs1bi")
        rs1b_out = dram.tile([D, HT], BF16, tag="rs1bo")
        dscr = dram.tile([32, 128, PREW], BF16, tag="dscr")

        # warmup collectives: without them the first real collective pays
        # ~100us of ncfw staging; with them RS1 is much cheaper.
        cw_in = dram.tile([128, 4], F32, tag="cwi")
        cw_out = dram.tile([128, 4], F32, tag="cwo")
        rw_in = dram.tile([4, 128], F32, tag="rwi")
        rw_out = dram.tile([1, 128], F32, tag="rwo")
        cw_sb = consts.tile([128, 4], F32)
        rw_sb = consts.tile([4, 128], F32)
        nc.vector.memset(cw_sb, 0.0)
        nc.vector.memset(rw_sb, 0.0)
        nc.sync.dma_start(out=cw_in, in_=cw_sb)
        nc.sync.dma_start(out=rw_in, in_=rw_sb)
        nc.gpsimd.collective_compute(
            "AllReduce", mybir.AluOpType.add, replica_groups=RG,
            ins=[cw_in[:]], outs=[cw_out[:]])
        nc.gpsimd.collective_compute(
            "ReduceScatter", mybir.AluOpType.add, replica_groups=RG,
            ins=[rw_in[:]], outs=[rw_out[:]])

        # one-time NEG fill of the dscr pad regions (left 153, right 159 cols)
        # via stride-0 re-reads of neg_sb; after this only the 1000 valid
        # columns are written per (head, chunk).
        padl = bass.AP(tensor=dscr.tensor, offset=dscr.offset,
                       ap=[[PREW, 128], [128 * PREW, 32], [1, PREOFF]])
        padr = bass.AP(tensor=dscr.tensor, offset=dscr.offset + PREOFF + 1000,
                       ap=[[PREW, 128], [128 * PREW, 32], [1, PREW - PREOFF - 1000]])
        negl = bass.AP(tensor=neg_sb.tensor, offset=neg_sb.offset,
                       ap=[neg_sb.ap[0], [0, 32], [1, PREOFF]])
        negr = bass.AP(tensor=neg_sb.tensor, offset=neg_sb.offset,
                       ap=[neg_sb.ap[0], [0, 32], [1, PREW - PREOFF - 1000]])
        nc.scalar.dma_start(out=padl, in_=negl)
        nc.scalar.dma_start(out=padr, in_=negr)

        with ExitStack() as astack:
            # attention-phase activations: freed before the FFN runs
            aacts = astack.enter_context(tc.tile_pool(name="aacts", bufs=1))
            qp = [aacts.tile([128, Q], BF16, tag=f"qp{i}", name=f"qp{i}") for i in range(2)]
            qpp = [aacts.tile([128, Q], BF16, tag=f"qpp{i}", name=f"qpp{i}") for i in range(2)]
            kk = [aacts.tile([128, K], BF16, tag=f"kk{i}", name=f"kk{i}") for i in range(2)]
            # vT[hf][hx]: transposed v for head 2*hf+hx, 64 channels + ones col
            vT = [[aacts.tile([128, K // 128, 80], BF16, tag=f"vT{i}{x}",
                              name=f"vT{i}{x}") for x in range(2)]
                  for i in range(2)]
            av = [aacts.tile([128, Q], BF16, tag=f"av{i}", name=f"av{i}") for i in range(2)]
            wo_sb = [aacts.tile([128, D], BF16, tag=f"wo{i}", name=f"wo{i}") for i in range(2)]
            bdp = astack.enter_context(tc.tile_pool(name="bdp", bufs=2))
            bde = {}

            def bd_prefetch(ic, eng):
                for hp in range(2):
                    t = bdp.tile([128, 2, SPAN], BF16, tag=f"bd{hp}", name="bd2")
                    idx = (2 * hp) * 8 + ic
                    diag = bass.AP(
                        tensor=dscr.tensor,
                        offset=dscr.offset + idx * 128 * PREW + RDOFF,
                        ap=[[PREW - 1, 128], [8 * 128 * PREW, 2], [1, SPAN]])
                    eng.dma_start(out=t, in_=diag)
                    bde[(ic, hp)] = t

            # ---------------- Phase 1: projections + BD pre ----------------
            with tc.tile_pool(name="p1w", bufs=2) as wp, \
                 tc.tile_pool(name="p1c", bufs=1) as cp, \
                 tc.tile_pool(name="p1ps", bufs=2, space="PSUM") as pp, \
                 tc.tile_pool(name="p1pp", bufs=2, space="PSUM") as ppre:
                cat_sb = [cp.tile([128, K], BF16, tag=f"cat{dc}", name=f"cat{dc}") for dc in range(8)]
                pos_sb = [cp.tile([128, 1000], BF16, tag=f"pos{dc}", name=f"pos{dc}") for dc in range(8)]
                rk = [cp.tile([128, 1000], BF16, tag=f"rk{i}", name=f"rk{i}") for i in range(2)]
                for dc in range(8):
                    weng = nc.gpsimd if dc % 2 == 0 else nc.scalar
                    weng.dma_start(out=cat_sb[dc], in_=cat[dc * 128:(dc + 1) * 128, :])
                # pos_w early on sync so r-proj isn't starved
                for dc in range(8):
                    nc.sync.dma_start(out=pos_sb[dc], in_=pos_w[dc * 128:(dc + 1) * 128, :])

                def proj_qk(wT, usrc, toff, tlen, is_q, dst=None, dst_split=None):
                    w_sb = [wp.tile([128, CO], BF16, tag=f"w1_{dc}", name="w1t") for dc in range(8)]
                    for dc in range(8):
                        weng = nc.sync if dc % 2 == 0 else nc.scalar
                        weng.dma_start(out=w_sb[dc], in_=wT[dc * 128:(dc + 1) * 128, :])
                    for oc in range(2):
                        for nb in range(tlen // 512):
                            ps = pp.tile([128, 512], F32, tag="ps1", name="ps1")
                            for dc in range(8):
                                nc.tensor.matmul(
                                    ps,
                                    lhsT=w_sb[dc][:, oc * 128:(oc + 1) * 128],
                                    rhs=cat_sb[dc][:, toff + nb * 512: toff + nb * 512 + 512],
                                    start=(dc == 0), stop=False)
                            uu = wp.tile([128, 512], BF16, tag="u1", name="uu", bufs=3)
                            nc.gpsimd.dma_start(
                                out=uu,
                                in_=usrc[oc * 128:(oc + 1) * 128, nb * 512:nb * 512 + 512])
                            nc.tensor.matmul(ps, lhsT=id_b, rhs=uu, start=False, stop=True)
                            sl = (slice(None), slice(nb * 512, nb * 512 + 512))
                            if is_q:
                                nc.vector.tensor_scalar_add(qp[oc][sl], ps, rwb_sb[:, oc:oc + 1])
                                nc.vector.tensor_scalar_add(qpp[oc][sl], ps, rrb_sb[:, oc:oc + 1])
                            elif dst_split is not None:
                                nc.scalar.copy(dst_split[oc][0][(slice(0, 64),) + sl[1:]],
                                               ps[0:64, :])
                                nc.scalar.copy(dst_split[oc][1][(slice(0, 64),) + sl[1:]],
                                               ps[64:128, :])
                            else:
                                nc.scalar.copy(dst[oc][sl], ps)

                # q projection first (BD pre depends on it)
                proj_qk(wqT, u_q, M, Q, True)

                # r_k projection
                wr_sb = [wp.tile([128, CO], BF16, tag=f"w1_{dc}", name="wrt") for dc in range(8)]
                for dc in range(8):
                    weng = nc.sync if dc % 2 == 0 else nc.scalar
                    weng.dma_start(out=wr_sb[dc], in_=wrT[dc * 128:(dc + 1) * 128, :])
                for oc in range(2):
                    for nb in range(2):
                        ps = pp.tile([128, 512], F32, tag="ps1", name="ps1")
                        for dc in range(8):
                            nc.tensor.matmul(
                                ps[:, :500],
                                lhsT=wr_sb[dc][:, oc * 128:(oc + 1) * 128],
                                rhs=pos_sb[dc][:, nb * 500:nb * 500 + 500],
                                start=(dc == 0), stop=(dc == 7))
                        nc.scalar.copy(rk[oc][:, nb * 500:nb * 500 + 500], ps[:, :500])

                # BD pre-matrices in attention consumption order.
                for ic in CORDER:
                    i0 = 128 * ic
                    for h in range(NHL):
                        ht, hh = h // 2, (h % 2) * 64
                        hsla = slice(hh, hh + 64)
                        P = ppre.tile([128, 1024], F32, tag="ppre", name="P")
                        for nb in range(2):
                            nc.tensor.matmul(
                                P[:, nb * 512:nb * 512 + 500],
                                lhsT=qpp[ht][hsla, i0:i0 + 128],
                                rhs=rk[ht][hsla, nb * 500:nb * 500 + 500],
                                start=True, stop=True)
                        pre = wp.tile([128, 1000], BF16, tag="pre", name="pre", bufs=3)
                        ceng = nc.scalar if (ic % 2 == 0) else nc.vector
                        oeng = nc.vector if (ic % 2 == 0) else nc.scalar
                        ceng.copy(pre[:, 0:500], P[:, 0:500]) if ceng is nc.scalar \
                            else ceng.tensor_copy(pre[:, 0:500], P[:, 0:500])
                        oeng.copy(pre[:, 500:1000], P[:, 512:1012]) if oeng is nc.scalar \
                            else oeng.tensor_copy(pre[:, 500:1000], P[:, 512:1012])
                        idx = h * 8 + ic
                        dst = bass.AP(tensor=dscr.tensor,
                                      offset=dscr.offset + idx * 128 * PREW + PREOFF,
                                      ap=[[PREW, 128], [1, 1000]])
                        deng = nc.gpsimd if h % 2 == 0 else nc.sync
                        deng.dma_start(out=dst, in_=pre)

                # k projection
                proj_qk(wkT, u_k, 0, K, False, dst=kk)

                # v projection into 80-partition per-head tiles (64 channels
                # + ones row at partition 64), then contiguous xbar-transpose
                # so the ones row becomes the rsum column of vT.
                vv = [[cp.tile([80, K], BF16, tag=f"vv{i}{x}", name=f"vv{i}{x}")
                       for x in range(2)] for i in range(2)]
                for hf in range(2):
                    for hx in range(2):
                        nc.vector.memset(vv[hf][hx][64:65, :], 1.0)
                proj_qk(wvT, u_v, 0, K, False, dst_split=vv)
                for hf in range(2):
                    for hx in range(2):
                        nc.sync.dma_start_transpose(vT[hf][hx], vv[hf][hx][0:80, :])
                for cc in range(2):
                    nc.scalar.dma_start(out=wo_sb[cc], in_=woT[cc * 128:(cc + 1) * 128, :])
                # prefetch bd (head-pairs) for the first chunk on scalar
                bd_prefetch(CORDER[0], nc.scalar)

            # ---------------- Phase 2: attention (S^T orientation) ----------
            with tc.tile_pool(name="p2pb", bufs=3) as pbp, \
                 tc.tile_pool(name="p2ri", bufs=2) as rip, \
                 tc.tile_pool(name="p2ot", bufs=2) as otp, \
                 tc.tile_pool(name="p2st", bufs=2, space="PSUM") as ppS, \
                 tc.tile_pool(name="p2av", bufs=2, space="PSUM") as ppA:

                # w1 prefetch early on gpsimd
                for dc in range(8):
                    nc.gpsimd.dma_start(out=w1_sb[dc], in_=wff1T[dc * 128:(dc + 1) * 128, :])

                def front(ic, h, bd2):
                    """S^T build + exp for one (head, chunk)."""
                    ht, hh = h // 2, (h % 2) * 64
                    hsla = slice(hh, hh + 64)
                    i0 = 128 * ic
                    st = ppS.tile([128, NJC, 128], F32, tag="st", name="st")
                    for c in range(NJC):
                        nc.tensor.matmul(
                            st[:, c, :],
                            lhsT=bd2[:, h % 2, c * 128:(c + 1) * 128],
                            rhs=id_b,
                            start=(c % 4 == 0), stop=False,
                            skip_group_check=True)
                        nc.tensor.matmul(
                            st[:, c, :],
                            lhsT=kk[ht][hsla, i0 + c * 128:i0 + (c + 1) * 128],
                            rhs=qp[ht][hsla, i0:i0 + 128],
                            start=False, stop=(c % 4 == 3 or c == NJC - 1),
                            skip_group_check=True)
                    probT = pbp.tile([128, NJC, 128], BF16, tag="pb", name="probT")
                    nc.scalar.activation(out=probT, in_=st, func=AF.Exp, scale=0.125)
                    if DEBUG and ic == 0 and h == 0:
                        nc.scalar.dma_start(out=dbg_bd, in_=bd2)
                        nc.scalar.dma_start(out=dbg_pt, in_=probT)
                    return probT

                def back(ic, h, probT, rinv2):
                    """AV + rsum extraction for one (head, chunk)."""
                    ht, hh = h // 2, (h % 2) * 64
                    i0 = 128 * ic
                    AVp = ppA.tile([65, 128], F32, tag="avp", name="AVp")
                    for c in range(NJC):
                        nc.tensor.matmul(AVp,
                                         lhsT=vT[ht][h % 2][:, ic + c, 0:65],
                                         rhs=probT[:, c, :],
                                         start=(c == 0), stop=(c == NJC - 1))
                    nc.vector.tensor_copy(av[ht][hh:hh + 64, i0:i0 + 128], AVp[0:64, :])
                    with nc.allow_low_precision(reason="softmax denom to bf16"):
                        nc.vector.reciprocal(rinv2[h % 2][0:1, ht, :],
                                             AVp[64:65, :])
                    if DEBUG and ic == 0 and h == 0:
                        dtile = pbp.tile([65, 128], F32, tag="dbgavp",
                                         name="dbgavp")
                        nc.vector.tensor_copy(dtile, AVp)
                        nc.scalar.dma_start(out=dbg_avp, in_=dtile)
                        nc.scalar.dma_start(out=dbg_ri, in_=rinv2[0])

                def chunk_tail(gi, ic, rinv2):
                    """normalize av, o_net partials, rs write, collectives."""
                    i0 = 128 * ic
                    # rinv broadcast: rows 0:64 <- head 2ht, 64:128 <- 2ht+1
                    rbc = ppS.tile([128, NJC, 128], F32, tag="st", name="rbc")
                    ones64 = sel2_sb[0:1, 0:64]
                    for ht in range(2):
                        nc.tensor.matmul(rbc[0:64, ht, :], lhsT=ones64,
                                         rhs=rinv2[0][0:1, ht, :],
                                         start=True, stop=False,
                                         skip_group_check=True)
                        nc.tensor.matmul(rbc[64:128, ht, :], lhsT=ones64,
                                         rhs=rinv2[1][0:1, ht, :],
                                         start=True, stop=(ht == 1),
                                         skip_group_check=True)
                    for ht in range(2):
                        nc.vector.tensor_mul(av[ht][:, i0:i0 + 128],
                                             av[ht][:, i0:i0 + 128], rbc[:, ht, :])
                    # o_net for this token chunk (8 oc blocks, 2 psum banks)
                    op = ppS.tile([128, NJC, 128], F32, tag="st", name="op")
                    for oc in range(8):
                        for cc in range(2):
                            nc.tensor.matmul(
                                op[:, oc, :],
                                lhsT=wo_sb[cc][:, oc * 128:(oc + 1) * 128],
                                rhs=av[cc][:, i0:i0 + 128],
                                start=(cc == 0 and oc % 4 == 0),
                                stop=(cc == 1 and oc % 4 == 3),
                                skip_group_check=True)
                    ot = otp.tile([128, 8, 128], BF16, tag="ot", name="ot")
                    nc.vector.tensor_copy(ot, op[:, 0:8, :])
                    g, half = ic // 2, ic % 2
                    rs_in = rs1a_in if half == 0 else rs1b_in
                    dst = rs_in[g].rearrange("(o p) t -> p o t", p=128)
                    nc.sync.dma_start(out=dst, in_=ot)
                    if gi == 3:
                        nc.gpsimd.collective_compute(
                            "ReduceScatter", mybir.AluOpType.add, replica_groups=RG,
                            ins=[rs1a_in[:]], outs=[rs1a_out[:]])
                    if gi == 7:
                        nc.gpsimd.collective_compute(
                            "ReduceScatter", mybir.AluOpType.add, replica_groups=RG,
                            ins=[rs1b_in[:]], outs=[rs1b_out[:]])

                def do_back(item):
                    gi, ic, h, probT, rinv2 = item
                    back(ic, h, probT, rinv2)
                    if h == NHL - 1:
                        chunk_tail(gi, ic, rinv2)

                pend = []
                for gi, ic in enumerate(CORDER):
                    # prefetch next chunk's bd head-pairs
                    if gi + 1 < 8:
                        bd_prefetch(CORDER[gi + 1], nc.gpsimd)
                    rinv2 = (rip.tile([1, 2, 128], BF16, tag="riA", name="rinvA"),
                             rip.tile([1, 2, 128], BF16, tag="riB", name="rinvB"))
                    for h in range(NHL):
                        probT = front(ic, h, bde[(ic, h // 2)])
                        if pend:
                            do_back(pend.pop(0))
                        pend.append((gi, ic, h, probT, rinv2))
                while pend:
                    do_back(pend.pop(0))
                if DEBUG:
                    nc.scalar.dma_start(out=dbg_av, in_=av[0])
                    nc.scalar.dma_start(out=dbg_vtx, in_=vT[0][0])

        # small persistent activations for the FFN phase
        acts = stack.enter_context(tc.tile_pool(name="acts", bufs=1))
        hbf = acts.tile([128, 8, TL], BF16)       # bf16 LN output (FFN input)
        h2 = acts.tile([128, 8, TL], F32)         # LN output + bff2 (residual)

        # FFN w2 weights: loads overlap RS + LN1 + ff1 (SBUF freed by astack)
        w2p = stack.enter_context(tc.tile_pool(name="w2p", bufs=1))
        w2_sb = [w2p.tile([128, D], BF16, tag=f"w2_{mc}", name=f"w2_{mc}")
                 for mc in range(32)]
        for mc in range(32):
            weng = (nc.sync, nc.scalar, nc.gpsimd)[mc % 3]
            weng.dma_start(out=w2_sb[mc], in_=wff2T[mc * 128:(mc + 1) * 128, :])

        # ---------------- Phase 3: LN + token-local FFN (half-pipelined) ----
        def layer_norm(pool, psum_pool, x, t0, tl, out_f32, out_bf16):
            """x: (128, 8, >=t0+tl) f32 tile; normalizes tokens [t0, t0+tl)."""
            sl = (slice(None), slice(None), slice(t0, t0 + tl))
            xb16 = pool.tile([128, 8, tl], BF16, tag="lnxb", name="lnxb")
            nc.vector.tensor_copy(xb16, x[sl])
            sq = pool.tile([128, 8, tl], BF16, tag="lnsq", name="lnsq")
            nc.vector.tensor_mul(sq, xb16, xb16)
            Sp = psum_pool.tile([1, 2 * tl], F32, tag="lnps", name="lnps")
            for dc in range(8):
                nc.tensor.matmul(Sp[:, 0:tl], lhsT=ones_sb, rhs=xb16[:, dc, :],
                                 start=(dc == 0), stop=(dc == 7),
                                 skip_group_check=True)
            for dc in range(8):
                nc.tensor.matmul(Sp[:, tl:2 * tl], lhsT=ones_sb, rhs=sq[:, dc, :],
                                 start=(dc == 0), stop=(dc == 7),
                                 skip_group_check=True)
            st = pool.tile([1, 2 * tl], F32, tag="lnst", name="lnst")
            # st[0:tl] = -mean ; st[tl:2tl] = rstd
            nc.vector.tensor_scalar_mul(st[:, 0:tl], Sp[:, 0:tl], -1.0 / D)
            m2 = pool.tile([1, tl], F32, tag="lnm2", name="lnm2")
            nc.vector.tensor_scalar_mul(m2, Sp[:, tl:2 * tl], 1.0 / D)
            msq = pool.tile([1, tl], F32, tag="lnmsq", name="lnmsq")
            nc.vector.tensor_mul(msq, st[:, 0:tl], st[:, 0:tl])
            var = pool.tile([1, tl], F32, tag="lnvar", name="lnvar")
            nc.vector.tensor_sub(var, m2, msq)
            sd = pool.tile([1, tl], F32, tag="lnsd", name="lnsd")
            nc.scalar.activation(out=sd, in_=var, func=AF.Sqrt, bias=eps_sb, scale=1.0)
            nc.vector.reciprocal(st[:, tl:2 * tl], sd)
            bcp = psum_pool.tile([128, 2 * tl], F32, tag="lnbc", name="lnbc")
            nc.tensor.matmul(bcp, lhsT=onesr_sb, rhs=st, start=True, stop=True,
                             skip_group_check=True)
            nm = bass.AP(tensor=bcp.tensor, offset=bcp.offset,
                         ap=[bcp.ap[0], [0, 8], [1, tl]])
            rs = bass.AP(tensor=bcp.tensor, offset=bcp.offset + tl,
                         ap=[bcp.ap[0], [0, 8], [1, tl]])
            cen = pool.tile([128, 8, tl], F32, tag="lncen", name="lncen")
            nc.vector.tensor_add(cen, x[sl], nm)
            if out_bf16 is not None:
                nc.vector.tensor_mul(out_bf16[sl], cen, rs)
            if out_f32 is not None:
                nc.vector.tensor_mul(out_f32[sl], cen, rs)

        with tc.tile_pool(name="p4s", bufs=1) as sp4, \
             tc.tile_pool(name="p4ln", bufs=1) as lnp, \
             tc.tile_pool(name="p4ps", bufs=1, space="PSUM") as pp4, \
             tc.tile_pool(name="p5ps", bufs=4, space="PSUM") as pp5, \
             tc.tile_pool(name="p5ps2", bufs=2, space="PSUM") as pp52:
            xat = sp4.tile([128, 8, TL], F32, tag="xat")
            ffh = sp4.tile([128, 32, TL], BF16, tag="ffh")
            for half in range(2):
                t0 = half * HT
                sl = (slice(None), slice(None), slice(t0, t0 + HT))
                rs_out = rs1a_out if half == 0 else rs1b_out
                xb = lnp.tile([128, 8, HT], BF16, tag="xb", name="xb")
                nc.sync.dma_start(out=xb,
                                  in_=rs_out.rearrange("(c p) t -> p c t", p=128))
                zs = lnp.tile([128, 8, HT], F32, tag="zs", name="zs")
                nc.sync.dma_start(
                    out=zs,
                    in_=zres[:, t0:t0 + HT].rearrange("(c p) t -> p c t", p=128))
                xf32 = lnp.tile([128, 8, HT], F32, tag="xf32", name="xf32")
                nc.vector.tensor_copy(xf32, xb)
                nc.vector.tensor_add(xat[sl], xf32, zs)
                if DEBUG:
                    nc.scalar.dma_start(out=dbg_xat[:, :, t0:t0 + HT],
                                        in_=xat[:, :, t0:t0 + HT])
                layer_norm(lnp, pp4, xat, t0, HT, h2, hbf)
                for dc in range(8):
                    nc.vector.tensor_scalar_add(h2[:, dc, t0:t0 + HT],
                                                h2[:, dc, t0:t0 + HT],
                                                bff2_sb[:, dc:dc + 1])
                for mc in range(32):
                    ps = pp5.tile([128, HT], F32, tag="ps5a", name="ps5a")
                    for dc in range(8):
                        nc.tensor.matmul(ps,
                                         lhsT=w1_sb[dc][:, mc * 128:(mc + 1) * 128],
                                         rhs=hbf[:, dc, t0:t0 + HT],
                                         start=(dc == 0), stop=(dc == 7))
                    nc.scalar.activation(out=ffh[:, mc, t0:t0 + HT], in_=ps,
                                         func=AF.Relu,
                                         bias=bff1_sb[:, mc:mc + 1], scale=1.0)

            xf = sp4.tile([128, 8, TL], F32, tag="xf")
            for oc in range(8):
                ps = pp52.tile([128, TL], F32, tag="ps5b", name="ps5b")
                for mc in range(32):
                    nc.tensor.matmul(ps,
                                     lhsT=w2_sb[mc][:, oc * 128:(oc + 1) * 128],
                                     rhs=ffh[:, mc, :],
                                     start=(mc == 0), stop=(mc == 31))
                nc.vector.tensor_add(xf[:, oc, :], ps, h2[:, oc, :])

            # ---------------- Phase 4: final LN (in place) + output --------
            for half in range(2):
                layer_norm(lnp, pp4, xf, half * HT, HT, xf, None)
            nc.sync.dma_start(out=y_out.rearrange("(c p) t -> p c t", p=128),
                              in_=xf)

    nc.compile()
    return nc


def _stage(z, z_hist, u, pos_emb, W_qkv, W_r, r_w_bias, r_r_bias, W_o, b_o,
           W_ff1, b_ff1, W_ff2, b_ff2):
    f32 = np.float32
    bf16 = ml_dtypes.bfloat16
    cats = [np.ascontiguousarray(
        np.concatenate([z_hist[b], z[b]], axis=1)).astype(bf16) for b in range(B)]
    pos_w = np.ascontiguousarray(pos_emb[0, :, 1048:2048]).astype(bf16)
    ident_bf = np.eye(128).astype(bf16)
    ones_bv = np.ones((128, 1)).astype(bf16)
    onesr_v = np.ones((1, 128), dtype=f32)
    sel2_v = np.zeros((2, 128), dtype=f32)
    sel2_v[0, 0:64] = 1.0
    sel2_v[1, 64:128] = 1.0
    sel2_v = sel2_v.astype(bf16)
    wff1T_full = np.ascontiguousarray(W_ff1.T).astype(bf16)
    wff2T_full = np.ascontiguousarray(W_ff2.T).astype(bf16)
    bff1_full = np.ascontiguousarray(b_ff1.reshape(32, 128).T, dtype=f32)
    bff2_full = np.ascontiguousarray(b_ff2.reshape(8, 128).T, dtype=f32)
    in_maps = []
    for c in range(N_CORES):
        b, g = c // 4, c % 4
        cs = 256 * g
        m = dict(
            cat=cats[b],
            wqT=np.ascontiguousarray(W_qkv[cs:cs + CO, :].T).astype(bf16),
            wkT=np.ascontiguousarray(W_qkv[1024 + cs:1024 + cs + CO, :].T).astype(bf16),
            wvT=np.ascontiguousarray(W_qkv[2048 + cs:2048 + cs + CO, :].T).astype(bf16),
            wrT=np.ascontiguousarray(W_r[cs:cs + CO, :].T).astype(bf16),
            u_q=np.ascontiguousarray(u[b, cs:cs + CO, M:]).astype(bf16),
            u_k=np.ascontiguousarray(u[b, 1024 + cs:1024 + cs + CO, :]).astype(bf16),
            u_v=np.ascontiguousarray(u[b, 2048 + cs:2048 + cs + CO, :]).astype(bf16),
            pos_w=pos_w,
            ident_b=ident_bf,
            ones_b=ones_bv,
            onesr=onesr_v,
            rwb=np.ascontiguousarray(
                r_w_bias[4 * g:4 * g + 4].reshape(CO).reshape(2, 128).T, dtype=f32),
            rrb=np.ascontiguousarray(
                r_r_bias[4 * g:4 * g + 4].reshape(CO).reshape(2, 128).T, dtype=f32),
            sel2=sel2_v,
            woT=np.ascontiguousarray(W_o[:, cs:cs + CO].T).astype(bf16),
            zres=np.ascontiguousarray(z[b, :, TL * g:TL * g + TL] + b_o[:, None], dtype=f32),
            wff1T=wff1T_full,
            bff1=bff1_full,
            wff2T=wff2T_full,
            bff2=bff2_full,
        )
        in_maps.append(m)
    return in_maps


def kernel(**inputs):
    if "nc" not in _CACHE:
        _CACHE["nc"] = _build()
    nc = _CACHE["nc"]
    in_maps = _stage(**inputs)
    res = bass_utils.run_bass_kernel_spmd(
        nc, in_maps, core_ids=list(range(N_CORES)))
    y = np.zeros((B, D, Q), dtype=np.float32)
    for c in range(N_CORES):
        b, g = c // 4, c % 4
        y[b, :, TL * g:TL * g + TL] = res.results[c]["y"]
    return y
